# revision 31
# baseline (speedup 1.0000x reference)
"""Multi-head attention (B=2, S=2048, H=32, D=128) on 8 Trainium2 NeuronCores.

Sharding: tensor-parallel across heads (core c owns heads [4c, 4c+4)).
Each core projects q/k/v for all 4096 tokens (weights column-sharded by
head), runs attention for its 4 heads, reshards the context head-major ->
token-major with one AllToAll per head, and runs the full output projection
on its 512-token block, producing o^T [4096, 512] (host transposes).

Scheduling: the tensor engine is the bottleneck (back-to-back 512-free
matmuls issue at ~259 ns; ~213 ns streaming + ~46 ns weight-load/sem
overhead), so the kernel is one continuous PE stream:
  P0 ~32 dummy matmuls warm the PE HAM clock (1.2 -> 2.4 GHz) while the
     first weight/activation DMAs land; a tiny AllToAll absorbs the ~108us
     collective warmup barrier.
  P1/P2 k then q projection, 512-token chunks, 2x[128,1024] PSUM,
     ScalarE evacuates.  (Sharing a stationary weight across two matmuls
     was measured to NOT reduce the 259 ns issue gap -- the overhead is
     per-instruction, not weight-load.)
  P3 v-proj for BOTH batches, dense.
  P4 attention in HEAD-MAJOR order (head h: b0 qb0-3, b1 qb0-3), blocks
     software-pipelined (ctx + softmax tail of block n-1 woven into the
     scores of block n).  AllToAll for head h fires right after its last
     block's ctx lands -- ~76us into attention for head 0 -- so all four
     A2As complete far ahead of their o-proj bursts: G0 runs after block
     23 (~129us of attention cover for the slow first A2A), G1-G3 after
     the final flush, by which time every A2A has landed.  The ctxg
     gather DMAs are issued from the GPSIMD queue (which serializes on
     the collectives anyway): a sem-blocked DMA stalls its whole engine
     FIFO, and on the Sync queue that jammed the ctxs writes feeding the
     attention pipeline for the full A2A latency.
PSUM: 2x[128,1024] + 4x[128,512] pools = exactly 8 banks.
"""

import numpy as np
import ml_dtypes

import concourse.bacc as bacc
import concourse.mybir as mybir
import concourse.tile as tile
from concourse.bass_utils import run_bass_kernel_spmd

bf16 = ml_dtypes.bfloat16

B, S, H, D = 2, 2048, 32, 128
DM = H * D                      # 4096
BT = B * S                      # 4096 tokens total
N_CORES = 8
HL = H // N_CORES               # heads per core = 4
FL = HL * D                     # feature slice per core = 512
TB = BT // N_CORES              # output token block per core = 512
NKT = S // 128                  # 16 k token-tiles per sequence
SCALE = float(D) ** -0.5

F32 = mybir.dt.float32
BF16 = mybir.dt.bfloat16
Exp = mybir.ActivationFunctionType.Exp
Copy = mybir.ActivationFunctionType.Copy

_CACHE = {}


def _build():
    nc = bacc.Bacc("TRN2", target_bir_lowering=False, debug=False,
                   num_devices=N_CORES)

    qT = nc.dram_tensor("qT", [DM, BT], BF16, kind="ExternalInput")
    kT = nc.dram_tensor("kT", [DM, BT], BF16, kind="ExternalInput")
    vT = nc.dram_tensor("vT", [DM, BT], BF16, kind="ExternalInput")
    wqT = nc.dram_tensor("wqT", [DM, FL], BF16, kind="ExternalInput")
    wkT = nc.dram_tensor("wkT", [DM, FL], BF16, kind="ExternalInput")
    wvT = nc.dram_tensor("wvT", [DM, FL], BF16, kind="ExternalInput")
    woT = nc.dram_tensor("woT", [DM, DM], BF16, kind="ExternalInput")
    outT = nc.dram_tensor("outT", [DM, TB], BF16, kind="ExternalOutput")

    v_r = vT.ap().rearrange("(kk p) t -> p kk t", p=128)
    wo_r = woT.ap().rearrange("(kk p) f -> p kk f", p=128)
    out_r = outT.ap().rearrange("(fo p) t -> p fo t", p=128)

    with tile.TileContext(nc) as tc:
        with (
            tc.tile_pool(name="persist", bufs=1) as persist,
            tc.tile_pool(name="big", bufs=2, space="PSUM") as bigp,
            tc.tile_pool(name="sm", bufs=4, space="PSUM") as smp,
            tc.tile_pool(name="dram", bufs=1, space="DRAM") as dram,
        ):
            qpT = persist.tile([128, HL, BT], BF16, tag="qpT")
            kpT = persist.tile([128, HL, BT], BF16, tag="kpT")
            vp = persist.tile([128, B * NKT, FL], BF16, tag="vp")
            ones_m = persist.tile([128, 128], BF16, tag="ones_m")
            nc.vector.memset(ones_m[:], 1.0)

            in_bufs, out_bufs = [], []
            for h in range(HL):
                in_bufs.append(dram.tile([N_CORES, 128, TB], BF16,
                                         tag=f"ain{h}", name=f"a2a_in{h}"))
                out_bufs.append(dram.tile([N_CORES, 128, TB], BF16,
                                          tag=f"aout{h}", name=f"a2a_out{h}"))

            # Tiny dummy AllToAll issued up front: the first collective pays
            # ~108us of warmup barrier; absorb it here, overlapped with P1.
            warm_in = dram.tile([N_CORES, 16], BF16, tag="warm_in",
                                name="warm_in")
            warm_out = dram.tile([N_CORES, 16], BF16, tag="warm_out",
                                 name="warm_out")
            nc.gpsimd.collective_compute(
                "AllToAll", mybir.AluOpType.bypass,
                replica_groups=[list(range(N_CORES))],
                ins=[warm_in.opt()], outs=[warm_out.opt()])

            # P0: dummy matmuls (no DMA dependency) keep the PE busy from
            # ~t0 so the HAM activity window un-throttles the clock before
            # the first real matmul's inputs land.
            warm_ps = smp.tile([128, 512], F32, tag="sm", name="warm_ps")
            for _ in range(48):
                nc.tensor.matmul(warm_ps[:, 0:128], ones_m[:], ones_m[:],
                                 start=True, stop=True)

            # ---------------- P1/P2: k then q projection (feature-major) ---
            with (
                tc.tile_pool(name="wkq", bufs=2) as wkq,
                tc.tile_pool(name="xkq", bufs=2) as xkq,
            ):
                for first, (x_dram, w_dram, out_t) in enumerate(
                        ((kT, wkT, kpT), (qT, wqT, qpT))):
                    first = first == 0
                    x_r = x_dram.ap().rearrange("(kk p) t -> p kk t", p=128)
                    w_r = w_dram.ap().rearrange("(kk p) f -> p kk f", p=128)
                    wh = []
                    pre_xs = None
                    for kh in range(2):
                        w = wkq.tile([128, 16, FL], BF16, tag="w", name="w")
                        if first and kh == 0:
                            # Interleave small first pieces of w and x so
                            # the first matmul starts after ~1 MB of DMA.
                            pre_xs = xkq.tile([128, 16, 512], BF16, tag="xs")
                            for lo, hi in ((0, 4), (4, 8), (8, 12),
                                           (12, 16)):
                                nc.sync.dma_start(out=w[:, lo:hi, :],
                                                  in_=w_r[:, lo:hi, :])
                                nc.sync.dma_start(out=pre_xs[:, lo:hi, :],
                                                  in_=x_r[:, lo:hi, 0:512])
                        else:
                            nc.sync.dma_start(
                                out=w[:], in_=w_r[:, kh * 16:(kh + 1) * 16, :])
                        wh.append(w)
                    for tch in range(8):       # 512-token chunks
                        pss = [bigp.tile([128, 1024], F32, tag="big",
                                         name=f"pss{mp}") for mp in range(2)]
                        for kh in range(2):    # halves of the contraction
                            if first and tch == 0 and kh == 0:
                                xs = pre_xs
                            else:
                                xs = xkq.tile([128, 16, 512], BF16, tag="xs")
                                nc.sync.dma_start(
                                    out=xs[:],
                                    in_=x_r[:, kh * 16:(kh + 1) * 16,
                                            tch * 512:(tch + 1) * 512])
                            # kk-outer order for the very first chunk: its
                            # x/w arrive in kk-slices, and kk-outer gives
                            # 16 matmuls of slack per slice instead of 4.
                            if first and tch == 0 and kh == 0:
                                order = [(kk, ms) for kk in range(16)
                                         for ms in range(4)]
                            else:
                                order = [(kk, ms) for ms in range(4)
                                         for kk in range(16)]
                            for kk, ms in order:
                                dst = pss[ms // 2][:, (ms % 2) * 512:
                                                   (ms % 2 + 1) * 512]
                                nc.tensor.matmul(
                                    dst,
                                    wh[kh][:, kk, ms * 128:(ms + 1) * 128],
                                    xs[:, kk, :],
                                    start=(kh == 0 and kk == 0),
                                    stop=(kh == 1 and kk == 15))
                        for mp in range(2):
                            nc.scalar.activation(
                                out_t[:, 2 * mp:2 * mp + 2,
                                      tch * 512:(tch + 1) * 512],
                                pss[mp][:], Copy)

            with tc.tile_pool(name="attn", bufs=1) as attn:
                # -------- software-pipelined attention block machinery -----
                pend = [None]

                def attn_iter(cur):
                    """Emit scores+exp for block `cur`; weave in the softmax
                    tail and the ctx matmuls of the previous block."""
                    p = pend[0]

                    def ctx_pair(kt0):
                        if p["ps_c"] is None:
                            p["ps_c"] = smp.tile([128, TB], F32, tag="sm",
                                                 name="ps_c")
                        for kt in (kt0, kt0 + 1):
                            nc.tensor.matmul(
                                p["ps_c"][:],
                                vp[:, p["b"] * NKT + kt,
                                   p["hl"] * 128:(p["hl"] + 1) * 128],
                                p["pt"][:, kt, :],
                                start=(kt == 0), stop=(kt == NKT - 1))

                    def finish_sums():
                        sp = attn.tile([128, TB], BF16, tag="sp", bufs=2)
                        nc.vector.tensor_add(sp[:], p["sp2"][:, 0, :],
                                             p["sp2"][:, 1, :])
                        ps_b = smp.tile([128, TB], F32, tag="sm", name="ps_b")
                        nc.tensor.matmul(ps_b[:], ones_m[:], sp[:],
                                         start=True, stop=True)
                        rsb = attn.tile([128, TB], F32, tag="rsb", bufs=2)
                        nc.vector.reciprocal_approx_fast(rsb[:], ps_b[:])
                        p["rsb"] = rsb

                    def finish_ctx():
                        ctxs = attn.tile([128, TB], BF16, tag="ctxs", bufs=2)
                        nc.vector.tensor_tensor(ctxs[:], p["ps_c"][:],
                                                p["rsb"][:],
                                                op=mybir.AluOpType.mult)
                        nc.sync.dma_start(
                            out=in_bufs[p["hl"]][p["b"] * 4 + p["qb"]],
                            in_=ctxs[:])

                    if cur is None:            # final flush
                        if p is not None:
                            finish_sums()
                            for g in range(8):
                                ctx_pair(2 * g)
                            finish_ctx()
                            pend[0] = None
                        return

                    hl, b, qb = cur
                    qs = slice(b * S + qb * TB, b * S + (qb + 1) * TB)
                    pt = attn.tile([128, NKT, TB], BF16, tag="pt", bufs=2)
                    sp2 = attn.tile([128, 2, TB], BF16, tag="sp2", bufs=2)
                    for g in range(8):
                        st = bigp.tile([128, 1024], F32, tag="big")
                        for half in range(2):
                            kt = 2 * g + half
                            nc.tensor.matmul(
                                st[:, half * 512:(half + 1) * 512],
                                kpT[:, hl, b * S + kt * 128:
                                    b * S + (kt + 1) * 128],
                                qpT[:, hl, qs],
                                start=True, stop=True)
                        nc.scalar.activation(pt[:, 2 * g:2 * g + 2, :],
                                             st[:], Exp, scale=SCALE)
                        if p is not None and g >= 1:
                            ctx_pair(2 * (g - 1))
                        if g == 7 and p is not None:
                            ctx_pair(14)
                            finish_ctx()   # before add7 so DVE isn't blocked
                        if g == 1:
                            nc.vector.tensor_add(sp2[:], pt[:, 0:2, :],
                                                 pt[:, 2:4, :])
                            if p is not None:
                                finish_sums()
                        elif g > 1:
                            nc.vector.tensor_add(sp2[:], sp2[:],
                                                 pt[:, 2 * g:2 * g + 2, :])
                    pend[0] = {"hl": hl, "b": b, "qb": qb, "pt": pt,
                               "sp2": sp2, "rsb": None, "ps_c": None}

                # ---------------- P3: v-proj, both batches, dense ----------
                with (
                    tc.tile_pool(name="wvp", bufs=1) as wvp,
                    tc.tile_pool(name="xvp", bufs=4) as xvp,
                ):
                    # These DMAs are emitted here but queue right behind
                    # P2's last activation loads, so wv and the first v
                    # chunks land just as the PE reaches P3.
                    wv = wvp.tile([128, 32, FL], BF16, tag="wv")
                    wv_r = wvT.ap().rearrange("(kk p) f -> p kk f", p=128)
                    nc.sync.dma_start(out=wv[:, 0:16, :],
                                      in_=wv_r[:, 0:16, :])
                    nc.sync.dma_start(out=wv[:, 16:32, :],
                                      in_=wv_r[:, 16:32, :])

                    # Each 256-token chunk: 2 k-tile chains (32 matmuls into
                    # one PSUM bank each), split into 4 parts of 8 matmuls;
                    # chunk c+1's DMAs are issued mid-chunk-c for overlap.
                    chunk_parts = []
                    chunk_dmas = []
                    for c in range(16):
                        xh = [None, None]

                        def mk_dma(xh=xh, c=c):
                            def dma(kh):
                                xh[kh] = xvp.tile([128, 16, 256], BF16,
                                                  tag="xs", name="xs")
                                nc.sync.dma_start(
                                    out=xh[kh][:],
                                    in_=v_r[:, kh * 16:(kh + 1) * 16,
                                            c * 256:(c + 1) * 256])
                            return dma

                        def mk_parts(xh=xh, c=c):
                            psh = [None]
                            parts = []
                            for kt2 in range(2):
                                for part in range(4):
                                    def pstep(kt2=kt2, part=part, c=c,
                                              xh=xh, psh=psh):
                                        kh, k8 = part // 2, (part % 2) * 8
                                        if part == 0:
                                            psh[0] = smp.tile(
                                                [128, FL], F32, tag="sm",
                                                name="ps_v")
                                        ps = psh[0]
                                        for kk in range(k8, k8 + 8):
                                            nc.tensor.matmul(
                                                ps[:],
                                                xh[kh][:, kk, kt2 * 128:
                                                       (kt2 + 1) * 128],
                                                wv[:, kh * 16 + kk, :],
                                                start=(part == 0 and
                                                       kk == k8),
                                                stop=(part == 3 and
                                                      kk == k8 + 7))
                                        if part == 3:
                                            nc.scalar.activation(
                                                vp[:, c * 2 + kt2, :],
                                                ps[:], Copy)
                                    parts.append(pstep)
                            return parts

                        chunk_dmas.append(mk_dma())
                        chunk_parts.append(mk_parts())

                    # steady pipeline: chunk c's parts 0-2, then chunk c+1's
                    # DMAs, then chunk c's parts 3-7.
                    chunk_dmas[0](0)
                    chunk_dmas[0](1)
                    for c in range(16):
                        for fn in chunk_parts[c][0:3]:
                            fn()
                        if c + 1 < 16:
                            chunk_dmas[c + 1](0)
                            chunk_dmas[c + 1](1)
                        for fn in chunk_parts[c][3:8]:
                            fn()

                # ---------------- P4: head-major attention + A2A + o-proj --
                with tc.tile_pool(name="oproj", bufs=1) as op:
                    acc = op.tile([128, 16, 2, TB], BF16, tag="acc")
                    ctxg = [None] * HL

                    def load_ctxg(h, eng):
                        # shared 2-deep ring: group h reuses the buffer of
                        # group h-2, already consumed by burst h-2.  Queue
                        # choice is delicate: a sem-blocked DMA stalls its
                        # whole engine FIFO, and the Sync queue runs far
                        # ahead of the PE, so a Sync-queued gather that
                        # waits on an A2A jams the ctxs writes feeding the
                        # attention pipeline (GPSIMD's SW-DGE is ~30x too
                        # slow).  Loads 0/1 ride the Activation
                        # queue -- ScalarE reaches them at exp-pipeline
                        # pace, after their A2A completed -- and loads 2/3 ride Sync in the end
                        # phase, where nothing queues behind them.
                        ctxg[h] = op.tile([128, N_CORES, TB], BF16,
                                          tag="ctxg", bufs=2, name=f"ctxg{h}")
                        eng.dma_start(
                            out=ctxg[h][:],
                            in_=out_bufs[h].rearrange("j p t -> p j t"))

                    def collective(h):
                        nc.gpsimd.collective_compute(
                            "AllToAll", mybir.AluOpType.bypass,
                            replica_groups=[list(range(N_CORES))],
                            ins=[in_bufs[h].opt()],
                            outs=[out_bufs[h].opt()])

                    woc_q = []

                    def woc_load(h, fop):
                        # Activation-queue DGE: keeps these off the Sync
                        # queue so ctxg/ctxs transfers never block them.
                        woc = op.tile([128, N_CORES, 256], BF16,
                                      tag="woc", bufs=4, name="woc")
                        nc.scalar.dma_start(
                            out=woc[:],
                            in_=wo_r[:, h * N_CORES:(h + 1) * N_CORES,
                                     fop * 256:(fop + 1) * 256])
                        woc_q.append(woc)

                    def oproj_group(h, next_h=None):
                        """Partial chains for one A2A group: 16 fop x 2 sub.
                        woT rows are host-permuted so group h's 8 contraction
                        tiles are rows [h*1024, (h+1)*1024).  Weight tiles
                        ride a 3-deep ring with ~1-tile lookahead (late woc
                        arrivals were measured stalling the burst when the
                        loads contend with a concurrent AllToAll)."""
                        nload = len(woc_q)   # group-h fops already in flight
                        while nload < 3:
                            woc_load(h, nload)
                            nload += 1
                        for fop in range(16):
                            if nload < 16:
                                woc_load(h, nload)
                                nload += 1
                            elif next_h is not None and nload < 19:
                                woc_load(next_h, nload - 16)
                                nload += 1
                            woc = woc_q.pop(0)
                            for sub in range(2):
                                ps = smp.tile([128, TB], F32, tag="sm",
                                              name="ps_o")
                                for j in range(N_CORES):
                                    nc.tensor.matmul(
                                        ps[:],
                                        woc[:, j, sub * 128:(sub + 1) * 128],
                                        ctxg[h][:, j, :],
                                        start=(j == 0),
                                        stop=(j == N_CORES - 1))
                                a = acc[:, fop, sub, :]
                                if h == 0:
                                    nc.vector.tensor_copy(a, ps[:])
                                elif h < HL - 1:
                                    nc.vector.tensor_add(a, a, ps[:])
                                else:
                                    ot = op.tile([128, TB], BF16, tag="ot",
                                                 bufs=2, name="ot")
                                    nc.vector.tensor_add(ot[:], a, ps[:])
                                    nc.sync.dma_start(
                                        out=out_r[:, fop * 2 + sub, :],
                                        in_=ot[:])

                    # Head-major block order: head h covers blocks
                    # 8h..8h+7 = (h, b, qb) for b in 0,1 and qb in 0..3.
                    # Block n's ctx is woven into block n+1, so A2A(h) is
                    # emitted right after block 8(h+1); its o-proj burst
                    # gets >= 120us of attention cover.
                    # A2A(h) fires as soon as head h's last ctx lands
                    # (during block 8(h+1)); ctxg gathers are emitted 2+
                    # blocks after their A2A is observed complete so the
                    # Sync queue never blocks; o-proj bursts trail further.
                    for n in range(32):
                        hl, b, qb = n // 8, (n % 8) // 4, n % 4
                        attn_iter((hl, b, qb))
                        if n in (8, 16, 24):
                            collective(n // 8 - 1)
                        if n == 21:
                            load_ctxg(0, nc.scalar)
                        if n == 23:
                            woc_load(0, 0)
                            woc_load(0, 1)
                            woc_load(0, 2)
                        if n == 24:
                            oproj_group(0, next_h=1)
                        if n == 26:
                            load_ctxg(1, nc.scalar)
                        if n == 28:
                            # ctxg2 gather must dispatch before A2A3 starts:
                            # concurrent with the collective it was measured
                            # at ~21 GB/s (47us) instead of ~3us.
                            load_ctxg(2, nc.scalar)
                    attn_iter(None)
                    collective(3)
                    oproj_group(1, next_h=2)
                    load_ctxg(3, nc.sync)
                    oproj_group(2, next_h=3)
                    oproj_group(3)

    nc.compile()
    return nc


def _prep_inputs(q, k, v, Wq, Wk, Wv, Wo):
    """Host-side sharding: cast to bf16, transpose to feature-major, slice."""
    q, k, v = (np.asarray(x, dtype=np.float32) for x in (q, k, v))
    Wq, Wk, Wv, Wo = (np.asarray(x, dtype=np.float32)
                      for x in (Wq, Wk, Wv, Wo))
    qT = np.ascontiguousarray(q.reshape(BT, DM).astype(bf16).T)
    kT = np.ascontiguousarray(k.reshape(BT, DM).astype(bf16).T)
    vT = np.ascontiguousarray(v.reshape(BT, DM).astype(bf16).T)
    # woT rows permuted so contraction tile kk' = h*8 + j holds the global
    # feature tile kk = j*HL + h (group-contiguous for the kernel).
    woT = Wo.astype(bf16).T.reshape(N_CORES, HL, 128, DM)
    woT = np.ascontiguousarray(woT.transpose(1, 0, 2, 3).reshape(DM, DM))
    in_maps = []
    for c in range(N_CORES):
        sl = slice(c * FL, (c + 1) * FL)
        in_maps.append({
            "qT": qT, "kT": kT, "vT": vT,
            "wqT": np.ascontiguousarray(Wq[sl, :].astype(bf16).T),
            "wkT": np.ascontiguousarray(Wk[sl, :].astype(bf16).T),
            "wvT": np.ascontiguousarray(Wv[sl, :].astype(bf16).T),
            "woT": woT,
        })
    return in_maps


def run_spmd(inputs, trace=False):
    if "nc" not in _CACHE:
        _CACHE["nc"] = _build()
    nc = _CACHE["nc"]
    in_maps = _prep_inputs(**inputs)
    res = run_bass_kernel_spmd(nc, in_maps, core_ids=list(range(N_CORES)),
                               trace=trace)
    o = np.empty((BT, DM), dtype=np.float32)
    for c in range(N_CORES):
        o[c * TB:(c + 1) * TB, :] = res.results[c]["outT"].T
    return o.reshape(B, S, DM), res


def kernel(q, k, v, Wq, Wk, Wv, Wo):
    o, _ = run_spmd(dict(q=q, k=k, v=v, Wq=Wq, Wk=Wk, Wv=Wv, Wo=Wo))
    return o


# revision 33
# speedup vs baseline: 1.0021x; 1.0021x over previous
"""Multi-head attention (B=2, S=2048, H=32, D=128) on 8 Trainium2 NeuronCores.

Sharding: tensor-parallel across heads (core c owns heads [4c, 4c+4)).
Each core projects q/k/v for all 4096 tokens (weights column-sharded by
head), runs attention for its 4 heads, reshards the context head-major ->
token-major with one AllToAll per head, and runs the full output projection
on its 512-token block, producing o^T [4096, 512] (host transposes).

Scheduling: the tensor engine is the bottleneck (back-to-back 512-free
matmuls issue at ~259 ns; ~213 ns streaming + ~46 ns weight-load/sem
overhead), so the kernel is one continuous PE stream:
  P0 ~32 dummy matmuls warm the PE HAM clock (1.2 -> 2.4 GHz) while the
     first weight/activation DMAs land; a tiny AllToAll absorbs the ~108us
     collective warmup barrier.
  P1/P2 k then q projection, 512-token chunks, 2x[128,1024] PSUM,
     ScalarE evacuates.  (Sharing a stationary weight across two matmuls
     was measured to NOT reduce the 259 ns issue gap -- the overhead is
     per-instruction, not weight-load.)
  P3 v-proj for BOTH batches, dense.
  P4 attention in HEAD-MAJOR order (head h: b0 qb0-3, b1 qb0-3), blocks
     software-pipelined (ctx + softmax tail of block n-1 woven into the
     scores of block n).  AllToAll for head h fires right after its last
     block's ctx lands -- ~76us into attention for head 0 -- so all four
     A2As complete far ahead of their o-proj bursts: G0 runs after block
     23 (~129us of attention cover for the slow first A2A), G1-G3 after
     the final flush, by which time every A2A has landed.  The ctxg
     gather DMAs are issued from the GPSIMD queue (which serializes on
     the collectives anyway): a sem-blocked DMA stalls its whole engine
     FIFO, and on the Sync queue that jammed the ctxs writes feeding the
     attention pipeline for the full A2A latency.
PSUM: 2x[128,1024] + 4x[128,512] pools = exactly 8 banks.
"""

import numpy as np
import ml_dtypes

import concourse.bacc as bacc
import concourse.mybir as mybir
import concourse.tile as tile
from concourse.bass_utils import run_bass_kernel_spmd

bf16 = ml_dtypes.bfloat16

B, S, H, D = 2, 2048, 32, 128
DM = H * D                      # 4096
BT = B * S                      # 4096 tokens total
N_CORES = 8
HL = H // N_CORES               # heads per core = 4
FL = HL * D                     # feature slice per core = 512
TB = BT // N_CORES              # output token block per core = 512
NKT = S // 128                  # 16 k token-tiles per sequence
SCALE = float(D) ** -0.5

F32 = mybir.dt.float32
BF16 = mybir.dt.bfloat16
Exp = mybir.ActivationFunctionType.Exp
Copy = mybir.ActivationFunctionType.Copy

_CACHE = {}


def _build():
    nc = bacc.Bacc("TRN2", target_bir_lowering=False, debug=False,
                   num_devices=N_CORES)

    qT = nc.dram_tensor("qT", [DM, BT], BF16, kind="ExternalInput")
    kT = nc.dram_tensor("kT", [DM, BT], BF16, kind="ExternalInput")
    vT = nc.dram_tensor("vT", [DM, BT], BF16, kind="ExternalInput")
    wqT = nc.dram_tensor("wqT", [DM, FL], BF16, kind="ExternalInput")
    wkT = nc.dram_tensor("wkT", [DM, FL], BF16, kind="ExternalInput")
    wvT = nc.dram_tensor("wvT", [DM, FL], BF16, kind="ExternalInput")
    woT = nc.dram_tensor("woT", [DM, DM], BF16, kind="ExternalInput")
    outT = nc.dram_tensor("outT", [DM, TB], BF16, kind="ExternalOutput")

    v_r = vT.ap().rearrange("(kk p) t -> p kk t", p=128)
    wo_r = woT.ap().rearrange("(kk p) f -> p kk f", p=128)
    out_r = outT.ap().rearrange("(fo p) t -> p fo t", p=128)

    with tile.TileContext(nc) as tc:
        with (
            tc.tile_pool(name="persist", bufs=1) as persist,
            tc.tile_pool(name="big", bufs=2, space="PSUM") as bigp,
            tc.tile_pool(name="sm", bufs=4, space="PSUM") as smp,
            tc.tile_pool(name="dram", bufs=1, space="DRAM") as dram,
        ):
            qpT = persist.tile([128, HL, BT], BF16, tag="qpT")
            kpT = persist.tile([128, HL, BT], BF16, tag="kpT")
            vp = persist.tile([128, B * NKT, FL], BF16, tag="vp")
            ones_m = persist.tile([128, 128], BF16, tag="ones_m")
            nc.vector.memset(ones_m[:], 1.0)

            in_bufs, out_bufs = [], []
            for h in range(HL):
                in_bufs.append(dram.tile([N_CORES, 128, TB], BF16,
                                         tag=f"ain{h}", name=f"a2a_in{h}"))
                out_bufs.append(dram.tile([N_CORES, 128, TB], BF16,
                                          tag=f"aout{h}", name=f"a2a_out{h}"))

            # Tiny dummy AllToAll issued up front: the first collective pays
            # ~108us of warmup barrier; absorb it here, overlapped with P1.
            warm_in = dram.tile([N_CORES, 16], BF16, tag="warm_in",
                                name="warm_in")
            warm_out = dram.tile([N_CORES, 16], BF16, tag="warm_out",
                                 name="warm_out")
            nc.gpsimd.collective_compute(
                "AllToAll", mybir.AluOpType.bypass,
                replica_groups=[list(range(N_CORES))],
                ins=[warm_in.opt()], outs=[warm_out.opt()])

            # P0: dummy matmuls (no DMA dependency) keep the PE busy from
            # ~t0 so the HAM activity window un-throttles the clock before
            # the first real matmul's inputs land.
            warm_ps = smp.tile([128, 512], F32, tag="sm", name="warm_ps")
            for _ in range(24):
                nc.tensor.matmul(warm_ps[:, 0:128], ones_m[:], ones_m[:],
                                 start=True, stop=True)

            # ---------------- P1/P2: k then q projection (feature-major) ---
            with (
                tc.tile_pool(name="wkq", bufs=2) as wkq,
                tc.tile_pool(name="xkq", bufs=2) as xkq,
            ):
                for first, (x_dram, w_dram, out_t) in enumerate(
                        ((kT, wkT, kpT), (qT, wqT, qpT))):
                    first = first == 0
                    x_r = x_dram.ap().rearrange("(kk p) t -> p kk t", p=128)
                    w_r = w_dram.ap().rearrange("(kk p) f -> p kk f", p=128)
                    wh = []
                    pre_xs = [None, None]
                    for kh in range(2):
                        w = wkq.tile([128, 16, FL], BF16, tag="w", name="w")
                        if first:
                            # First chunk: w and x arrive kk-slice by
                            # kk-slice (kh0) / in small pieces (kh1), so
                            # kk-outer matmul consumption tracks DMA
                            # arrival ~1:1 from the very start.
                            pre_xs[kh] = xkq.tile([128, 16, 512], BF16,
                                                  tag="xs",
                                                  name=f"pre_xs{kh}")
                            if kh == 0:
                                for kk in range(16):
                                    nc.sync.dma_start(
                                        out=w[:, kk:kk + 1, :],
                                        in_=w_r[:, kk:kk + 1, :])
                                    nc.sync.dma_start(
                                        out=pre_xs[0][:, kk:kk + 1, :],
                                        in_=x_r[:, kk:kk + 1, 0:512])
                            else:
                                for lo, hi in ((0, 4), (4, 16)):
                                    nc.sync.dma_start(
                                        out=w[:, lo:hi, :],
                                        in_=w_r[:, 16 + lo:16 + hi, :])
                                    nc.sync.dma_start(
                                        out=pre_xs[1][:, lo:hi, :],
                                        in_=x_r[:, 16 + lo:16 + hi, 0:512])
                        else:
                            nc.sync.dma_start(
                                out=w[:], in_=w_r[:, kh * 16:(kh + 1) * 16, :])
                        wh.append(w)
                    for tch in range(8):       # 512-token chunks
                        pss = [bigp.tile([128, 1024], F32, tag="big",
                                         name=f"pss{mp}") for mp in range(2)]
                        for kh in range(2):    # halves of the contraction
                            if first and tch == 0:
                                xs = pre_xs[kh]
                            else:
                                xs = xkq.tile([128, 16, 512], BF16, tag="xs")
                                nc.sync.dma_start(
                                    out=xs[:],
                                    in_=x_r[:, kh * 16:(kh + 1) * 16,
                                            tch * 512:(tch + 1) * 512])
                            # kk-outer order for the very first chunk: its
                            # x/w arrive in kk-slices, and kk-outer gives
                            # 16 matmuls of slack per slice instead of 4.
                            if first and tch == 0:
                                order = [(kk, ms) for kk in range(16)
                                         for ms in range(4)]
                            else:
                                order = [(kk, ms) for ms in range(4)
                                         for kk in range(16)]
                            for kk, ms in order:
                                dst = pss[ms // 2][:, (ms % 2) * 512:
                                                   (ms % 2 + 1) * 512]
                                nc.tensor.matmul(
                                    dst,
                                    wh[kh][:, kk, ms * 128:(ms + 1) * 128],
                                    xs[:, kk, :],
                                    start=(kh == 0 and kk == 0),
                                    stop=(kh == 1 and kk == 15))
                        for mp in range(2):
                            nc.scalar.activation(
                                out_t[:, 2 * mp:2 * mp + 2,
                                      tch * 512:(tch + 1) * 512],
                                pss[mp][:], Copy)

            with tc.tile_pool(name="attn", bufs=1) as attn:
                # -------- software-pipelined attention block machinery -----
                pend = [None]

                def attn_iter(cur):
                    """Emit scores+exp for block `cur`; weave in the softmax
                    tail and the ctx matmuls of the previous block."""
                    p = pend[0]

                    def ctx_pair(kt0):
                        if p["ps_c"] is None:
                            p["ps_c"] = smp.tile([128, TB], F32, tag="sm",
                                                 name="ps_c")
                        for kt in (kt0, kt0 + 1):
                            nc.tensor.matmul(
                                p["ps_c"][:],
                                vp[:, p["b"] * NKT + kt,
                                   p["hl"] * 128:(p["hl"] + 1) * 128],
                                p["pt"][:, kt, :],
                                start=(kt == 0), stop=(kt == NKT - 1))

                    def finish_sums():
                        sp = attn.tile([128, TB], BF16, tag="sp", bufs=2)
                        nc.vector.tensor_add(sp[:], p["sp2"][:, 0, :],
                                             p["sp2"][:, 1, :])
                        ps_b = smp.tile([128, TB], F32, tag="sm", name="ps_b")
                        nc.tensor.matmul(ps_b[:], ones_m[:], sp[:],
                                         start=True, stop=True)
                        rsb = attn.tile([128, TB], F32, tag="rsb", bufs=2)
                        nc.vector.reciprocal_approx_fast(rsb[:], ps_b[:])
                        p["rsb"] = rsb

                    def finish_ctx():
                        ctxs = attn.tile([128, TB], BF16, tag="ctxs", bufs=2)
                        nc.vector.tensor_tensor(ctxs[:], p["ps_c"][:],
                                                p["rsb"][:],
                                                op=mybir.AluOpType.mult)
                        nc.sync.dma_start(
                            out=in_bufs[p["hl"]][p["b"] * 4 + p["qb"]],
                            in_=ctxs[:])

                    if cur is None:            # final flush
                        if p is not None:
                            finish_sums()
                            for g in range(8):
                                ctx_pair(2 * g)
                            finish_ctx()
                            pend[0] = None
                        return

                    hl, b, qb = cur
                    qs = slice(b * S + qb * TB, b * S + (qb + 1) * TB)
                    pt = attn.tile([128, NKT, TB], BF16, tag="pt", bufs=2)
                    sp2 = attn.tile([128, 2, TB], BF16, tag="sp2", bufs=2)
                    for g in range(8):
                        st = bigp.tile([128, 1024], F32, tag="big")
                        for half in range(2):
                            kt = 2 * g + half
                            nc.tensor.matmul(
                                st[:, half * 512:(half + 1) * 512],
                                kpT[:, hl, b * S + kt * 128:
                                    b * S + (kt + 1) * 128],
                                qpT[:, hl, qs],
                                start=True, stop=True)
                        nc.scalar.activation(pt[:, 2 * g:2 * g + 2, :],
                                             st[:], Exp, scale=SCALE)
                        if p is not None and g >= 1:
                            ctx_pair(2 * (g - 1))
                        if g == 7 and p is not None:
                            ctx_pair(14)
                            finish_ctx()   # before add7 so DVE isn't blocked
                        if g == 1:
                            nc.vector.tensor_add(sp2[:], pt[:, 0:2, :],
                                                 pt[:, 2:4, :])
                            if p is not None:
                                finish_sums()
                        elif g > 1:
                            nc.vector.tensor_add(sp2[:], sp2[:],
                                                 pt[:, 2 * g:2 * g + 2, :])
                    pend[0] = {"hl": hl, "b": b, "qb": qb, "pt": pt,
                               "sp2": sp2, "rsb": None, "ps_c": None}

                # ---------------- P3: v-proj, both batches, dense ----------
                with (
                    tc.tile_pool(name="wvp", bufs=1) as wvp,
                    tc.tile_pool(name="xvp", bufs=4) as xvp,
                ):
                    # These DMAs are emitted here but queue right behind
                    # P2's last activation loads, so wv and the first v
                    # chunks land just as the PE reaches P3.
                    wv = wvp.tile([128, 32, FL], BF16, tag="wv")
                    wv_r = wvT.ap().rearrange("(kk p) f -> p kk f", p=128)
                    nc.sync.dma_start(out=wv[:, 0:16, :],
                                      in_=wv_r[:, 0:16, :])
                    nc.sync.dma_start(out=wv[:, 16:32, :],
                                      in_=wv_r[:, 16:32, :])

                    # Each 256-token chunk: 2 k-tile chains (32 matmuls into
                    # one PSUM bank each), split into 4 parts of 8 matmuls;
                    # chunk c+1's DMAs are issued mid-chunk-c for overlap.
                    chunk_parts = []
                    chunk_dmas = []
                    for c in range(16):
                        xh = [None, None]

                        def mk_dma(xh=xh, c=c):
                            def dma(kh):
                                xh[kh] = xvp.tile([128, 16, 256], BF16,
                                                  tag="xs", name="xs")
                                nc.sync.dma_start(
                                    out=xh[kh][:],
                                    in_=v_r[:, kh * 16:(kh + 1) * 16,
                                            c * 256:(c + 1) * 256])
                            return dma

                        def mk_parts(xh=xh, c=c):
                            psh = [None]
                            parts = []
                            for kt2 in range(2):
                                for part in range(4):
                                    def pstep(kt2=kt2, part=part, c=c,
                                              xh=xh, psh=psh):
                                        kh, k8 = part // 2, (part % 2) * 8
                                        if part == 0:
                                            psh[0] = smp.tile(
                                                [128, FL], F32, tag="sm",
                                                name="ps_v")
                                        ps = psh[0]
                                        for kk in range(k8, k8 + 8):
                                            nc.tensor.matmul(
                                                ps[:],
                                                xh[kh][:, kk, kt2 * 128:
                                                       (kt2 + 1) * 128],
                                                wv[:, kh * 16 + kk, :],
                                                start=(part == 0 and
                                                       kk == k8),
                                                stop=(part == 3 and
                                                      kk == k8 + 7))
                                        if part == 3:
                                            nc.scalar.activation(
                                                vp[:, c * 2 + kt2, :],
                                                ps[:], Copy)
                                    parts.append(pstep)
                            return parts

                        chunk_dmas.append(mk_dma())
                        chunk_parts.append(mk_parts())

                    # steady pipeline: chunk c's parts 0-2, then chunk c+1's
                    # DMAs, then chunk c's parts 3-7.
                    chunk_dmas[0](0)
                    chunk_dmas[0](1)
                    for c in range(16):
                        for fn in chunk_parts[c][0:3]:
                            fn()
                        if c + 1 < 16:
                            chunk_dmas[c + 1](0)
                            chunk_dmas[c + 1](1)
                        for fn in chunk_parts[c][3:8]:
                            fn()

                # ---------------- P4: head-major attention + A2A + o-proj --
                with tc.tile_pool(name="oproj", bufs=1) as op:
                    acc = op.tile([128, 16, 2, TB], BF16, tag="acc")
                    ctxg = [None] * HL

                    def load_ctxg(h, eng):
                        # shared 2-deep ring: group h reuses the buffer of
                        # group h-2, already consumed by burst h-2.  Queue
                        # choice is delicate: a sem-blocked DMA stalls its
                        # whole engine FIFO, and the Sync queue runs far
                        # ahead of the PE, so a Sync-queued gather that
                        # waits on an A2A jams the ctxs writes feeding the
                        # attention pipeline (GPSIMD's SW-DGE is ~30x too
                        # slow).  Loads 0/1 ride the Activation
                        # queue -- ScalarE reaches them at exp-pipeline
                        # pace, after their A2A completed -- and loads 2/3 ride Sync in the end
                        # phase, where nothing queues behind them.
                        ctxg[h] = op.tile([128, N_CORES, TB], BF16,
                                          tag="ctxg", bufs=2, name=f"ctxg{h}")
                        eng.dma_start(
                            out=ctxg[h][:],
                            in_=out_bufs[h].rearrange("j p t -> p j t"))

                    def collective(h):
                        nc.gpsimd.collective_compute(
                            "AllToAll", mybir.AluOpType.bypass,
                            replica_groups=[list(range(N_CORES))],
                            ins=[in_bufs[h].opt()],
                            outs=[out_bufs[h].opt()])

                    woc_q = []

                    def woc_load(h, fop):
                        # Activation-queue DGE: keeps these off the Sync
                        # queue so ctxg/ctxs transfers never block them.
                        woc = op.tile([128, N_CORES, 256], BF16,
                                      tag="woc", bufs=4, name="woc")
                        nc.scalar.dma_start(
                            out=woc[:],
                            in_=wo_r[:, h * N_CORES:(h + 1) * N_CORES,
                                     fop * 256:(fop + 1) * 256])
                        woc_q.append(woc)

                    def oproj_group(h, next_h=None):
                        """Partial chains for one A2A group: 16 fop x 2 sub.
                        woT rows are host-permuted so group h's 8 contraction
                        tiles are rows [h*1024, (h+1)*1024).  Weight tiles
                        ride a 3-deep ring with ~1-tile lookahead (late woc
                        arrivals were measured stalling the burst when the
                        loads contend with a concurrent AllToAll)."""
                        nload = len(woc_q)   # group-h fops already in flight
                        while nload < 3:
                            woc_load(h, nload)
                            nload += 1
                        for fop in range(16):
                            if nload < 16:
                                woc_load(h, nload)
                                nload += 1
                            elif next_h is not None and nload < 19:
                                woc_load(next_h, nload - 16)
                                nload += 1
                            woc = woc_q.pop(0)
                            for sub in range(2):
                                ps = smp.tile([128, TB], F32, tag="sm",
                                              name="ps_o")
                                for j in range(N_CORES):
                                    nc.tensor.matmul(
                                        ps[:],
                                        woc[:, j, sub * 128:(sub + 1) * 128],
                                        ctxg[h][:, j, :],
                                        start=(j == 0),
                                        stop=(j == N_CORES - 1))
                                a = acc[:, fop, sub, :]
                                if h == 0:
                                    nc.vector.tensor_copy(a, ps[:])
                                elif h < HL - 1:
                                    nc.vector.tensor_add(a, a, ps[:])
                                else:
                                    ot = op.tile([128, TB], BF16, tag="ot",
                                                 bufs=2, name="ot")
                                    nc.vector.tensor_add(ot[:], a, ps[:])
                                    nc.sync.dma_start(
                                        out=out_r[:, fop * 2 + sub, :],
                                        in_=ot[:])

                    # Head-major block order: head h covers blocks
                    # 8h..8h+7 = (h, b, qb) for b in 0,1 and qb in 0..3.
                    # Block n's ctx is woven into block n+1, so A2A(h) is
                    # emitted right after block 8(h+1); its o-proj burst
                    # gets >= 120us of attention cover.
                    # A2A(h) fires as soon as head h's last ctx lands
                    # (during block 8(h+1)); ctxg gathers are emitted 2+
                    # blocks after their A2A is observed complete so the
                    # Sync queue never blocks; o-proj bursts trail further.
                    for n in range(32):
                        hl, b, qb = n // 8, (n % 8) // 4, n % 4
                        attn_iter((hl, b, qb))
                        if n in (8, 16, 24):
                            collective(n // 8 - 1)
                        if n == 21:
                            load_ctxg(0, nc.scalar)
                        if n == 23:
                            woc_load(0, 0)
                            woc_load(0, 1)
                            woc_load(0, 2)
                        if n == 24:
                            oproj_group(0, next_h=1)
                        if n == 26:
                            load_ctxg(1, nc.scalar)
                        if n == 28:
                            # ctxg2 gather must dispatch before A2A3 starts:
                            # concurrent with the collective it was measured
                            # at ~21 GB/s (47us) instead of ~3us.
                            load_ctxg(2, nc.scalar)
                    attn_iter(None)
                    collective(3)
                    oproj_group(1, next_h=2)
                    load_ctxg(3, nc.sync)
                    oproj_group(2, next_h=3)
                    oproj_group(3)

    nc.compile()
    return nc


def _prep_inputs(q, k, v, Wq, Wk, Wv, Wo):
    """Host-side sharding: cast to bf16, transpose to feature-major, slice."""
    q, k, v = (np.asarray(x, dtype=np.float32) for x in (q, k, v))
    Wq, Wk, Wv, Wo = (np.asarray(x, dtype=np.float32)
                      for x in (Wq, Wk, Wv, Wo))
    qT = np.ascontiguousarray(q.reshape(BT, DM).astype(bf16).T)
    kT = np.ascontiguousarray(k.reshape(BT, DM).astype(bf16).T)
    vT = np.ascontiguousarray(v.reshape(BT, DM).astype(bf16).T)
    # woT rows permuted so contraction tile kk' = h*8 + j holds the global
    # feature tile kk = j*HL + h (group-contiguous for the kernel).
    woT = Wo.astype(bf16).T.reshape(N_CORES, HL, 128, DM)
    woT = np.ascontiguousarray(woT.transpose(1, 0, 2, 3).reshape(DM, DM))
    in_maps = []
    for c in range(N_CORES):
        sl = slice(c * FL, (c + 1) * FL)
        in_maps.append({
            "qT": qT, "kT": kT, "vT": vT,
            "wqT": np.ascontiguousarray(Wq[sl, :].astype(bf16).T),
            "wkT": np.ascontiguousarray(Wk[sl, :].astype(bf16).T),
            "wvT": np.ascontiguousarray(Wv[sl, :].astype(bf16).T),
            "woT": woT,
        })
    return in_maps


def run_spmd(inputs, trace=False):
    if "nc" not in _CACHE:
        _CACHE["nc"] = _build()
    nc = _CACHE["nc"]
    in_maps = _prep_inputs(**inputs)
    res = run_bass_kernel_spmd(nc, in_maps, core_ids=list(range(N_CORES)),
                               trace=trace)
    o = np.empty((BT, DM), dtype=np.float32)
    for c in range(N_CORES):
        o[c * TB:(c + 1) * TB, :] = res.results[c]["outT"].T
    return o.reshape(B, S, DM), res


def kernel(q, k, v, Wq, Wk, Wv, Wo):
    o, _ = run_spmd(dict(q=q, k=k, v=v, Wq=Wq, Wk=Wk, Wv=Wv, Wo=Wo))
    return o
